# revision 1
# baseline (speedup 1.0000x reference)
"""LightGCN 3-layer propagation + batch dot on 8 Trainium2 NeuronCores.

Strategy: row-partition the 150K nodes across 8 cores (18816 rows each).
Per layer, each core gathers source embeddings for its edges via int16
dma_gather (3 address banks centered to cover 150528 rows), multiplies by
edge values, and segment-sums into 128-row PSUM tiles using one-hot
matmuls on the tensor engine. New embeddings are AllGathered between
layers. The final (acc/4, batch gather + dot) epilogue runs on host.
"""
import numpy as np

N_USERS = 100000
N_ITEMS = 50000
N = N_USERS + N_ITEMS        # 150000
D = 64
NCORES = 8
N_PAD = 150528               # 8 * 18816
R = N_PAD // NCORES          # 18816 rows per core
T = R // 128                 # 147 row-tiles per core
STRIP = 21                   # tiles per metadata strip (147 = 7*21)
BANK_BASE = (0, 32768, 65536, 98304, 131072)
BANK_LO = (0, 32768, 65536, 98304, 131072)

_compiled = {}


def _preprocess(edge_row, edge_col, edge_val):
    """Sort/pad edges into per-core fixed-capacity (tile, bank) segments.

    Returns (L, idx16 [8,128,C*8], valv [8,128,C], rlv [8,128,C]) where
    L = (L0, L1, L2) chunk capacities and C = T * (L0+L1+L2).
    """
    edge_row = np.asarray(edge_row).astype(np.int64)
    edge_col = np.asarray(edge_col).astype(np.int64)
    edge_val = np.asarray(edge_val).astype(np.float32)

    owner = edge_row // R
    tloc = (edge_row % R) >> 7
    rl = (edge_row & 127).astype(np.int16)
    NB = len(BANK_BASE)
    bank = np.searchsorted(np.asarray(BANK_LO), edge_col, side="right") - 1
    cidx = (edge_col - np.asarray(BANK_BASE)[bank]).astype(np.int16)

    seg = (owner * T + tloc) * NB + bank
    nseg = NCORES * T * NB
    counts = np.bincount(seg, minlength=nseg)
    # uniform per-bank chunk capacity across all cores/tiles
    cmax = counts.reshape(NCORES, T, NB).max(axis=(0, 1))
    L = tuple(int(-(-int(c) // 128)) for c in cmax)     # ceil/128
    LT = sum(L)
    C = T * LT

    order = np.argsort(seg, kind="stable")
    sseg = seg[order]
    starts = np.concatenate([[0], np.cumsum(counts)[:-1]])
    rank = np.arange(len(order)) - starts[sseg]

    # chunk base (in edges) of each segment inside its core's stream
    segL = np.concatenate([[0], np.cumsum(L)[:-1]])
    o = order
    core_o, tloc_o, bank_o = owner[o], tloc[o], bank[o]
    pos = (tloc_o * LT + segL[bank_o]) * 128 + rank     # within-core edge slot

    E_cap = C * 128
    # pad gathers hit row BANK_BASE[b] (valid, val=0); indices stay >= 0
    cidx_a = np.zeros((NCORES, E_cap), dtype=np.int16)
    val_a = np.zeros((NCORES, E_cap), dtype=np.float32)
    rl_a = np.zeros((NCORES, E_cap), dtype=np.int16)
    cidx_a[core_o, pos] = cidx[o]
    val_a[core_o, pos] = edge_val[o]
    rl_a[core_o, pos] = rl[o]

    # device layouts
    valv = val_a.reshape(NCORES, C, 128).transpose(0, 2, 1).copy()   # [8,128,C]
    rlv = rl_a.reshape(NCORES, C, 128).transpose(0, 2, 1).copy()     # [8,128,C]
    # idx16: per segment of len Lb chunks, wrapped [16, Lb*8]; tiled to 128 parts
    X = cidx_a.reshape(NCORES, T, LT * 128)
    idx16 = np.empty((NCORES, 128, C * 8), dtype=np.int16)
    for b in range(NB):
        c0, Lb = int(segL[b]), L[b]
        if Lb == 0:
            continue
        blk = X[:, :, c0 * 128:(c0 + Lb) * 128].reshape(NCORES, T, Lb * 8, 16)
        # dev[c, p, (t*LT+c0)*8 + j] = blk[c, t, j, p % 16]
        part16 = np.moveaxis(blk, 3, 1)                   # [8, 16, T, Lb*8]
        part = np.tile(part16, (1, 8, 1, 1))              # [8, 128, T, Lb*8]
        cols = ((np.arange(T) * LT + c0)[:, None] * 8 + np.arange(Lb * 8)[None, :])
        idx16[:, :, cols.ravel()] = part.reshape(NCORES, 128, -1)
    return L, idx16, valv, rlv


def _build(L, reps=1):
    import os as _os
    STAGE = int(_os.environ.get("STAGE", "6"))
    import concourse.bacc as bacc
    import concourse.bass as bass
    import concourse.mybir as mybir
    import concourse.tile as tile
    from concourse.library_config import mlp

    NB = len(L)
    LT = sum(L)
    C = T * LT
    f32 = mybir.dt.float32
    i16 = mybir.dt.int16

    nc = bacc.Bacc("TRN2", target_bir_lowering=False, debug=False,
                   num_devices=NCORES)
    table0 = nc.dram_tensor("table0", [N_PAD, D], f32, kind="ExternalInput")
    idx16 = nc.dram_tensor("idx16", [128, C * 8], i16, kind="ExternalInput")
    valv = nc.dram_tensor("valv", [128, C], f32, kind="ExternalInput")
    rlv = nc.dram_tensor("rlv", [128, C], i16, kind="ExternalInput")
    e0 = nc.dram_tensor("e0", [R, D], f32, kind="ExternalInput")
    out_acc = nc.dram_tensor("out_acc", [R, D], f32, kind="ExternalOutput")

    segc = [0]
    for x in L[:-1]:
        segc.append(segc[-1] + x)
    with tile.TileContext(nc, num_cores=NCORES) as tc:
        with tc.tile_pool(name="const", bufs=1) as constp, \
             tc.tile_pool(name="accp", bufs=1) as accp, \
             tc.tile_pool(name="meta", bufs=2) as metap, \
             tc.tile_pool(name="gp", bufs=3) as gp, \
             tc.tile_pool(name="sp", bufs=2) as sp, \
             tc.tile_pool(name="ob", bufs=4) as obp, \
             tc.tile_pool(name="psum", bufs=4, space="PSUM") as psp, \
             tc.tile_pool(name="dram", bufs=1, space="DRAM") as dram:
            nc.gpsimd.load_library(mlp)
            import os as _os2
            S_MODE = int(_os2.environ.get("S_MODE", "1"))
            iota = constp.tile([128, 1, 128], i16)
            nc.gpsimd.iota(iota[:, 0, :], pattern=[[1, 128]], base=0,
                           channel_multiplier=0)
            iotab = constp.tile([128, LT, 128], i16)
            nc.gpsimd.iota(iotab[:], pattern=[[0, LT], [1, 128]], base=0,
                           channel_multiplier=0)
            acc = accp.tile([128, T * D], f32)
            nc.sync.dma_start(out=acc[:].rearrange("p (t d) -> p t d", d=D),
                              in_=e0[:].rearrange("(t p) d -> p t d", p=128))

            tb1 = dram.tile([N_PAD, D], f32, tag="tb1")
            tb2 = dram.tile([N_PAD, D], f32, tag="tb2")
            sh1 = dram.tile([R, D], f32, tag="sh1")
            sh2 = dram.tile([R, D], f32, tag="sh2")
            tables = [table0[:], tb1[:], tb2[:]]
            shards = [sh1[:], sh2[:]]

            import contextlib
            loop_cm = tc.For_i(0, reps, 1) if reps > 1 else contextlib.nullcontext()
            with loop_cm:
              for layer in range(3 if STAGE >= 2 else 0):
                src = tables[layer]
                for s in range(T // STRIP):
                    ixs = metap.tile([128, STRIP * LT * 8], i16, tag="ixs")
                    vls = metap.tile([128, STRIP * LT], f32, tag="vls")
                    rls = metap.tile([128, STRIP * LT], i16, tag="rls")
                    c0s = s * STRIP * LT
                    nc.sync.dma_start(out=ixs[:], in_=idx16[:, c0s * 8:(c0s + STRIP * LT) * 8])
                    nc.sync.dma_start(out=vls[:], in_=valv[:, c0s:c0s + STRIP * LT])
                    nc.sync.dma_start(out=rls[:], in_=rlv[:, c0s:c0s + STRIP * LT])
                    for tt in range(STRIP):
                        t = s * STRIP + tt
                        ps = psp.tile([128, D], f32)
                        S = sp.tile([128, LT, 128], f32, tag="S")
                        if STAGE >= 2:
                            if S_MODE == 1:
                                nc.vector.tensor_tensor(
                                    out=S[:],
                                    in0=rls[:, tt * LT:(tt + 1) * LT].to_broadcast([128, LT, 128]),
                                    in1=iota[:].to_broadcast([128, LT, 128]),
                                    op=mybir.AluOpType.is_equal)
                            else:
                                nc.vector.tensor_tensor(
                                    out=S[:],
                                    in0=rls[:, tt * LT:(tt + 1) * LT].to_broadcast([128, LT, 128]),
                                    in1=iotab[:],
                                    op=mybir.AluOpType.is_equal)
                        for b in range(NB):
                            Lb = L[b]
                            if Lb == 0:
                                continue
                            g = gp.tile([128, Lb, D], f32, tag=f"g{b}")
                            ib = (tt * LT + segc[b]) * 8
                            if STAGE >= 3: nc.gpsimd.dma_gather(
                                g[:], src[BANK_BASE[b]:, :],
                                ixs[:, ib:ib + Lb * 8], Lb * 128, Lb * 128, D, single_packet=False)
                            vb = tt * LT + segc[b]
                            if STAGE >= 3: nc.vector.tensor_tensor(
                                out=g[:],
                                in0=vls[:, vb:vb + Lb].to_broadcast([128, Lb, D]),
                                in1=g[:],
                                op=mybir.AluOpType.mult)
                            for k in range(Lb if STAGE >= 4 else 0):
                                kk = segc[b] + k
                                nc.tensor.matmul(
                                    out=ps[:], lhsT=S[:, kk, :], rhs=g[:, k, :],
                                    start=(kk == 0), stop=(kk == LT - 1))
                        if STAGE >= 4: nc.vector.tensor_add(out=acc[:, t * D:(t + 1) * D],
                                             in0=acc[:, t * D:(t + 1) * D],
                                             in1=ps[:])
                        if layer < 2 and STAGE >= 5:
                            ob = obp.tile([128, D], f32, tag="ob")
                            nc.scalar.copy(out=ob[:], in_=ps[:])
                            nc.sync.dma_start(
                                out=shards[layer].rearrange("(t p) d -> p t d", p=128)[:, t, :],
                                in_=ob[:])
                if layer < 2 and STAGE >= 5:
                    if STAGE == 5 or _os.environ.get("NOCC"):
                        nc.gpsimd.dma_start(out=tables[layer + 1][:R], in_=shards[layer][:])
                    else:
                        nc.gpsimd.collective_compute(
                            "AllGather", mybir.AluOpType.bypass,
                            replica_groups=[list(range(NCORES))],
                            ins=[shards[layer]], outs=[tables[layer + 1]])
            nc.sync.dma_start(out=out_acc[:].rearrange("(t p) d -> p t d", p=128),
                              in_=acc[:].rearrange("p (t d) -> p t d", d=D))
    nc.compile()
    return nc


def kernel(user_emb, item_emb, edge_row, edge_col, edge_val, users, items):
    from concourse.bass_utils import run_bass_kernel_spmd

    table0 = np.zeros((N_PAD, D), dtype=np.float32)
    table0[:N_USERS] = np.asarray(user_emb, dtype=np.float32)
    table0[N_USERS:N] = np.asarray(item_emb, dtype=np.float32)

    L, idx16, valv, rlv = _preprocess(edge_row, edge_col, edge_val)
    if L not in _compiled:
        _compiled[L] = _build(L)
    nc = _compiled[L]

    in_maps = []
    for c in range(NCORES):
        in_maps.append({
            "table0": table0,
            "idx16": idx16[c],
            "valv": valv[c],
            "rlv": rlv[c],
            "e0": table0[c * R:(c + 1) * R],
        })
    res = run_bass_kernel_spmd(nc, in_maps, core_ids=list(range(NCORES)))
    acc = np.concatenate([res.results[c]["out_acc"] for c in range(NCORES)], axis=0)

    users = np.asarray(users).astype(np.int64)
    items = np.asarray(items).astype(np.int64)
    ue = acc[users]
    ie = acc[N_USERS + items]
    gamma = np.sum(ue * ie, axis=1) / np.float32(16.0)
    return gamma.astype(np.float32)



# revision 2
# speedup vs baseline: 3.5390x; 3.5390x over previous
"""LightGCN 3-layer propagation + batch dot on 8 Trainium2 NeuronCores.

Transfer-diet rewrite of the original one-hot-matmul kernel. The per-call
device invocation previously shipped ~590MB over the host link (full node
table replicated to all 8 cores, 8x-replicated gather indices, f32
metadata, 38.5MB output + donated zeros); this version ships ~94MB:
  (a) node table AllGathered on device from per-core f32 shards,
  (b) gather indices sent un-replicated [16, C*8] and replicated
      16->128 partitions by 8 on-device DMAs per strip,
  (c) per-edge one-hot row ids sent as uint8, converted on device,
  (d) the batch dot epilogue computed on device (AllGather acc ->
      banked dma_gather of the 8192 batch rows -> dma_scatter_add into
      slot order -> elementwise dot) so D2H is 16KB per core.
All arithmetic stays f32 (fp16 val/product variants pass but with only
1.4x margin against the 2e-2 rel-err gate; f32 gives 360x).

Per layer (unchanged from the original scheme): row-partitioned nodes,
per (128-row tile, col-bank) fixed-capacity edge chunks, dma_gather of
source embeddings (col-sorted within chunks for DRAM locality), val
multiply, one-hot f32 matmuls segment-summing into f32 PSUM.
DGE note: num_idxs_reg must equal the count of non-negative indices, so
all index streams are padded with valid indices (row 0 for gathers, dump
rows past slot 8191 for the epilogue scatter).
"""
import numpy as np

N_USERS = 100000
N_ITEMS = 50000
N = N_USERS + N_ITEMS        # 150000
D = 64
NCORES = 8
N_PAD = 150528               # 8 * 18816
R = N_PAD // NCORES          # 18816 rows per core
T = R // 128                 # 147 row-tiles per core
STRIP = 21                   # tiles per metadata strip (147 = 7*21)
NB = 5                       # 32768-row col banks
BANK_BASE = (0, 32768, 65536, 98304, 131072)
BATCH = 4096
# epilogue per-bank gather capacities (rows 0..150527, bank = row>>15)
EPC_MIN = (1536, 1536, 1536, 3072, 1792)

_compiled = {}


def _preprocess(edge_row, edge_col, edge_val):
    """Sort/pad edges into per-core fixed-capacity (tile, bank) chunks.

    Returns (L, idxc [8,16,C*8] i16, valv [8,128,C] f16, rlv [8,128,C] i16)
    where L = per-bank chunk capacities (in 128-edge units) and C = T*sum(L).
    Within each chunk edges are sorted by column index for gather locality.
    """
    er = np.asarray(edge_row).astype(np.int64)
    ec = np.asarray(edge_col).astype(np.int64)
    ev = np.asarray(edge_val).astype(np.float32)

    owner = er // R
    rrem = er - owner * R
    tloc = rrem >> 7
    rl = (rrem & 127).astype(np.uint8)
    bank = ec >> 15
    cidx = (ec & 32767).astype(np.int16)

    seg = (owner * T + tloc) * NB + bank
    nseg = NCORES * T * NB
    counts = np.bincount(seg, minlength=nseg)
    cmax = counts.reshape(NCORES, T, NB).max(axis=(0, 1))
    L = tuple(int(-(-int(c) // 128)) for c in cmax)     # ceil/128
    LT = sum(L)
    C = T * LT

    order = np.argsort(seg * 32768 + cidx.astype(np.int64))
    sseg = seg[order]
    starts = np.concatenate([[0], np.cumsum(counts)[:-1]])
    rank = np.arange(len(order)) - starts[sseg]

    segL = np.concatenate([[0], np.cumsum(L)[:-1]])
    core_o, tloc_o, bank_o = owner[order], tloc[order], bank[order]
    pos = (tloc_o * LT + segL[bank_o]) * 128 + rank     # within-core edge slot

    E_cap = C * 128
    # pad slots: idx 0 (valid row, gathers garbage), val 0, rl 0 -> adds 0
    cidx_a = np.zeros((NCORES, E_cap), dtype=np.int16)
    val_a = np.zeros((NCORES, E_cap), dtype=np.float32)
    rl_a = np.zeros((NCORES, E_cap), dtype=np.uint8)
    cidx_a[core_o, pos] = cidx[order]
    val_a[core_o, pos] = ev[order]
    rl_a[core_o, pos] = rl[order]

    idxc = cidx_a.reshape(NCORES, C * 8, 16).transpose(0, 2, 1).copy()  # [8,16,C*8]
    valv = val_a.reshape(NCORES, C, 128).transpose(0, 2, 1).copy()      # [8,128,C]
    rlv = rl_a.reshape(NCORES, C, 128).transpose(0, 2, 1).copy()        # [8,128,C]
    return L, idxc, valv, rlv


def _ep_meta(users, items):
    """Banked gather + slot-scatter indices for the batch-dot epilogue.

    Slot s in [0,4096) is user s; slot 4096+s is item s. Returns
    (caps, gidx [128, G/16] i16, sidx [128, G/16] i16) with G = sum(caps).
    All indices are valid (num_idxs_reg must equal the count of
    non-negative indices): gather pads hit row 0 of the bank, scatter
    pads land in the 128 dump rows past the 8192 real slots.
    """
    rows = np.concatenate([
        np.asarray(users).astype(np.int64),
        N_USERS + np.asarray(items).astype(np.int64),
    ])
    bank = rows >> 15
    cidx = (rows & 32767).astype(np.int16)
    order = np.argsort(bank, kind="stable")
    counts = np.bincount(bank, minlength=NB)
    caps = tuple(
        int(max(EPC_MIN[b], -(-int(counts[b]) // 128) * 128)) for b in range(NB)
    )
    G = sum(caps)
    gidx = np.zeros(G, dtype=np.int16)
    sidx = (2 * BATCH + (np.arange(G) & 127)).astype(np.int16)  # dump rows
    off = 0
    p = 0
    for b in range(NB):
        cnt = int(counts[b])
        sel = order[p:p + cnt]
        p += cnt
        gidx[off:off + cnt] = cidx[sel]
        sidx[off:off + cnt] = sel.astype(np.int16)      # slot id
        off += caps[b]
    gw = np.tile(gidx.reshape(G // 16, 16).T, (8, 1)).copy()  # [128, G/16]
    sw = np.tile(sidx.reshape(G // 16, 16).T, (8, 1)).copy()
    return caps, gw, sw


def _build(L, EPC):
    import concourse.bacc as bacc
    import concourse.mybir as mybir
    import concourse.tile as tile
    from concourse.library_config import mlp

    LT = sum(L)
    C = T * LT
    G = sum(EPC)
    f32 = mybir.dt.float32
    i16 = mybir.dt.int16
    u8 = mybir.dt.uint8

    EP, IX, CC = 3, 1, 7   # production path (debug bisect knobs, hardcoded)

    nc = bacc.Bacc("TRN2", target_bir_lowering=False, debug=False,
                   num_devices=NCORES)
    e0 = nc.dram_tensor("e0", [R, D], f32, kind="ExternalInput")
    idxc = nc.dram_tensor("idxc", [16 if IX else 128, C * 8], i16,
                          kind="ExternalInput")
    valv = nc.dram_tensor("valv", [128, C], f32, kind="ExternalInput")
    rlv = nc.dram_tensor("rlv", [128, C], u8, kind="ExternalInput")
    egi = nc.dram_tensor("egi", [128, G // 16], i16, kind="ExternalInput")
    esi = nc.dram_tensor("esi", [128, G // 16], i16, kind="ExternalInput")
    outg = nc.dram_tensor("outg", [BATCH], f32, kind="ExternalOutput")

    segc = [0]
    for x in L[:-1]:
        segc.append(segc[-1] + x)
    RG = [list(range(NCORES))]

    with tile.TileContext(nc, num_cores=NCORES) as tc:
        with tc.tile_pool(name="const", bufs=1) as constp, \
             tc.tile_pool(name="accp", bufs=1) as accp, \
             tc.tile_pool(name="psum", bufs=4, space="PSUM") as psp, \
             tc.tile_pool(name="dram", bufs=1, space="DRAM") as dram:
            nc.gpsimd.load_library(mlp)
            iota = constp.tile([128, 1, 128], i16)
            nc.gpsimd.iota(iota[:, 0, :], pattern=[[1, 128]], base=0,
                           channel_multiplier=0)
            acc = accp.tile([128, T * D], f32)
            nc.sync.dma_start(out=acc[:].rearrange("p (t d) -> p t d", d=D),
                              in_=e0[:].rearrange("(t p) d -> p t d", p=128))

            tb0 = dram.tile([N_PAD, D], f32, tag="tb0")
            tb1 = dram.tile([N_PAD, D], f32, tag="tb1")
            tb2 = dram.tile([N_PAD, D], f32, tag="tb2")
            sh1 = dram.tile([R, D], f32, tag="sh1")
            sh2 = dram.tile([R, D], f32, tag="sh2")
            accd = dram.tile([R, D], f32, tag="accd")
            accf = dram.tile([N_PAD, D], f32, tag="accf")
            ues = dram.tile([2 * BATCH + 128, D], f32, tag="ues")  # +dump rows

            # full node table from per-core shards (replaces 308MB H2D).
            # Collectives cannot read IO tensors -> stage through sh0.
            sh0 = dram.tile([R, D], f32, tag="sh0")
            nc.sync.dma_start(out=sh0[:], in_=e0[:])
            if CC & 1:
                nc.gpsimd.collective_compute("AllGather", mybir.AluOpType.bypass,
                                             replica_groups=RG,
                                             ins=[sh0[:]], outs=[tb0[:]])
            else:
                nc.gpsimd.dma_start(out=tb0[:][:R], in_=sh0[:])
            # zero the epilogue scatter target early (overlaps with layers)
            zt = constp.tile([128, 2 * BATCH // 128, D], f32)
            nc.vector.memset(zt[:], 0)
            nc.sync.dma_start(
                out=ues[:][0:2 * BATCH, :].rearrange("(c p) d -> p c d", p=128),
                in_=zt[:])
            egs = constp.tile([128, G // 16], i16)
            nc.sync.dma_start(out=egs[:], in_=egi[:])
            ess = constp.tile([128, G // 16], i16)
            nc.sync.dma_start(out=ess[:], in_=esi[:])

            tables = [tb0, tb1, tb2]
            shards = [sh1, sh2]

            with tc.tile_pool(name="meta", bufs=2) as metap, \
                 tc.tile_pool(name="gp", bufs=3) as gp, \
                 tc.tile_pool(name="sp", bufs=2) as sp, \
                 tc.tile_pool(name="ob", bufs=4) as obp:
                for layer in range(3):
                    src = tables[layer][:]
                    for s in range(T // STRIP):
                        cols = STRIP * LT
                        c0s = s * cols
                        ixs = metap.tile([128, cols * 8], i16, tag="ixs")
                        if IX:
                            for a in range(8):
                                nc.sync.dma_start(
                                    out=ixs[16 * a:16 * (a + 1), :],
                                    in_=idxc[:, c0s * 8:(c0s + cols) * 8])
                        else:
                            nc.sync.dma_start(
                                out=ixs[:],
                                in_=idxc[:, c0s * 8:(c0s + cols) * 8])
                        vls = metap.tile([128, cols], f32, tag="vls")
                        nc.sync.dma_start(out=vls[:], in_=valv[:, c0s:c0s + cols])
                        rls8 = metap.tile([128, cols], u8, tag="rls8")
                        nc.sync.dma_start(out=rls8[:], in_=rlv[:, c0s:c0s + cols])
                        rls = metap.tile([128, cols], i16, tag="rls")
                        nc.scalar.copy(out=rls[:], in_=rls8[:])
                        for tt in range(STRIP):
                            t = s * STRIP + tt
                            ps = psp.tile([128, D], f32)
                            S = sp.tile([128, LT, 128], f32, tag="S")
                            nc.vector.tensor_tensor(
                                out=S[:],
                                in0=rls[:, tt * LT:(tt + 1) * LT].to_broadcast([128, LT, 128]),
                                in1=iota[:].to_broadcast([128, LT, 128]),
                                op=mybir.AluOpType.is_equal)
                            for b in range(NB):
                                Lb = L[b]
                                if Lb == 0:
                                    continue
                                g = gp.tile([128, Lb, D], f32, tag=f"g{b}")
                                ib = (tt * LT + segc[b]) * 8
                                nc.gpsimd.dma_gather(
                                    g[:], src[BANK_BASE[b]:, :],
                                    ixs[:, ib:ib + Lb * 8], Lb * 128, Lb * 128,
                                    D, single_packet=False)
                                vb = tt * LT + segc[b]
                                nc.vector.tensor_tensor(
                                    out=g[:],
                                    in0=vls[:, vb:vb + Lb].to_broadcast([128, Lb, D]),
                                    in1=g[:],
                                    op=mybir.AluOpType.mult)
                                for k in range(Lb):
                                    kk = segc[b] + k
                                    nc.tensor.matmul(
                                        out=ps[:], lhsT=S[:, kk, :], rhs=g[:, k, :],
                                        start=(kk == 0), stop=(kk == LT - 1))
                            nc.vector.tensor_add(out=acc[:, t * D:(t + 1) * D],
                                                 in0=acc[:, t * D:(t + 1) * D],
                                                 in1=ps[:])
                            if layer < 2:
                                ob = obp.tile([128, D], f32, tag="ob")
                                nc.scalar.copy(out=ob[:], in_=ps[:])
                                nc.sync.dma_start(
                                    out=shards[layer][:].rearrange(
                                        "(t p) d -> p t d", p=128)[:, t, :],
                                    in_=ob[:])
                        del ixs, vls, rls8, rls
                    if layer < 2:
                        if CC & 2:
                            nc.gpsimd.collective_compute(
                                "AllGather", mybir.AluOpType.bypass,
                                replica_groups=RG,
                                ins=[shards[layer][:]], outs=[tables[layer + 1][:]])
                        else:
                            nc.gpsimd.dma_start(out=tables[layer + 1][:][:R],
                                                in_=shards[layer][:])

            # ---- epilogue: gamma[s] = (acc[u_s] . acc[N_USERS+i_s]) / 16 ----
            nc.sync.dma_start(out=accd[:].rearrange("(t p) d -> p t d", p=128),
                              in_=acc[:].rearrange("p (t d) -> p t d", d=D))
            if CC & 4:
                nc.gpsimd.collective_compute("AllGather", mybir.AluOpType.bypass,
                                             replica_groups=RG,
                                             ins=[accd[:]], outs=[accf[:]])
            else:
                nc.gpsimd.dma_start(out=accf[:][:R], in_=accd[:])
            with tc.tile_pool(name="ep", bufs=1) as epp:
                half = BATCH // 128
                if EP >= 0:
                    goff = 0
                    for b in range(NB):
                        cap = EPC[b]
                        if cap == 0:
                            continue
                        gb = epp.tile([128, cap // 128, D], f32, tag=f"eg{b}")
                        if EP >= 1:
                            nc.gpsimd.dma_gather(
                                gb[:], accf[:][BANK_BASE[b]:, :],
                                egs[:, goff // 16:(goff + cap) // 16], cap, cap,
                                D, single_packet=False)
                        if EP >= 2:
                            nc.gpsimd.dma_scatter_add(
                                ues[:], gb[:],
                                ess[:, goff // 16:(goff + cap) // 16], cap, cap,
                                D, single_packet=False)
                        goff += cap
                    ue_sb = epp.tile([128, 2 * BATCH // 128, D], f32)
                    nc.sync.dma_start(
                        out=ue_sb[:],
                        in_=ues[:][0:2 * BATCH, :].rearrange("(c p) d -> p c d", p=128))
                    prod = epp.tile([128, half, D], f32)
                    nc.vector.tensor_tensor(out=prod[:], in0=ue_sb[:, 0:half, :],
                                            in1=ue_sb[:, half:2 * half, :],
                                            op=mybir.AluOpType.mult)
                    gm = epp.tile([128, half], f32)
                    nc.vector.tensor_reduce(out=gm[:], in_=prod[:],
                                            axis=mybir.AxisListType.X,
                                            op=mybir.AluOpType.add)
                    gms = epp.tile([128, half], f32)
                    nc.vector.tensor_scalar_mul(out=gms[:], in0=gm[:],
                                                scalar1=1.0 / 16.0)
                else:
                    gms = epp.tile([128, half], f32)
                    nc.vector.memset(gms[:], 0)
                nc.sync.dma_start(out=outg[:].rearrange("(c p) -> p c", p=128),
                                  in_=gms[:])
    nc.compile()
    return nc


def kernel(user_emb, item_emb, edge_row, edge_col, edge_val, users, items):
    from concourse.bass_utils import run_bass_kernel_spmd

    e0_full = np.zeros((N_PAD, D), dtype=np.float32)
    e0_full[:N_USERS] = np.asarray(user_emb, dtype=np.float32)
    e0_full[N_USERS:N] = np.asarray(item_emb, dtype=np.float32)

    L, idxc, valv, rlv = _preprocess(edge_row, edge_col, edge_val)
    caps, gw, sw = _ep_meta(users, items)
    key = (L, caps)
    if key not in _compiled:
        _compiled[key] = _build(L, caps)
    nc = _compiled[key]

    in_maps = []
    for c in range(NCORES):
        in_maps.append({
            "e0": e0_full[c * R:(c + 1) * R],
            "idxc": idxc[c],
            "valv": valv[c],
            "rlv": rlv[c],
            "egi": gw,
            "esi": sw,
        })
    res = run_bass_kernel_spmd(nc, in_maps, core_ids=list(range(NCORES)))
    return np.asarray(res.results[0]["outg"], dtype=np.float32)
